# revision 18
# baseline (speedup 1.0000x reference)
"""Trainium2 Bass kernel for BilinearScoringFunction.

scores[b] = relu( einsum('bi,hij,bj->bh', head, W_R, tail)
                  + concat(head, tail) @ V_R.T + b_R ) @ u_R

B=4096, D=512, H=256. Sharded over 8 NeuronCores along the hidden dim H
(32 hidden units per core); each core computes partial u_R dot products
over its hidden slice, and the host sums the 8 partial score vectors.

All matmul operands are bf16 (PSUM accumulation fp32): fp32r stationary
loads take ~218ns/128-col and gate the PE at ~263ns/matmul; bf16 loads
(~107ns) hide under the 512-col streaming time (~215ns), so the PE runs
at the streaming roofline. The tail-side multiply+reduce stays fp32
(VectorE TTR reads fp32 PSUM; tail tile kept fp32 in SBUF).

Per core structure:
  phase 2 (dominant): per h: T_h = head @ W_h on TensorE (4 accumulating
    K=128 matmuls, N=512), then one fused VectorE custom-DVE
    TENSOR_TENSOR_REDUCE computes bil[:, h] = rowsum(T_h * tail) straight
    out of PSUM.
  linear term (inserted into the PE stream after h=5): h-major
    lin^T = V_slice @ concat^T as 8 accumulating matmuls per 512-batch
    tile with the tiny V chunks stationary, bias added in PSUM, then
    32x32 VectorE block transposes back to b-major.
  phase 3: per batch tile: bil + lin (VectorE), relu (ScalarE),
    fused dot with u_slice (VectorE custom-DVE reduce).

DMA order: W_0 first, then head^T/tail as interleaved per-batch-tile
slices, so the first matmul quad gates on ~0.8MB and h=0/h=1 run
DMA-paced as the streams land.
"""

import os
from contextlib import ExitStack

import numpy as np
import ml_dtypes

import concourse.bacc as bacc
import concourse.tile as tile
import concourse.mybir as mybir
from concourse import bass_utils
from concourse.dve_ops import TENSOR_TENSOR_REDUCE

B, D, H = 4096, 512, 256
NCORES = 8
HSL = H // NCORES          # hidden units per core = 32
P = 128                    # partitions
BT = B // P                # batch tiles of 128 = 32
NB5 = B // 512             # batch tiles of 512 = 8
KD = D // P                # contraction chunks per operand = 4
KC = 2 * KD                # concat contraction chunks = 8
LIN_AT_H = 6               # insert linear-term matmuls before this h
FP8_HS = (8, 14, 20, 26)   # h's computed in fp8 e4m3 DoubleRow (2x PE rate)

_F32 = mybir.dt.float32
_BF16 = mybir.dt.bfloat16
_F8 = mybir.dt.float8e4

_NC_CACHE = None


def _build_nc():
    nc = bacc.Bacc(
        "TRN2",
        target_bir_lowering=False,
        debug=False,
        enable_asserts=False,
        num_devices=NCORES,
    )
    # all pre-arranged host-side so every DMA is a clean 2D/3D copy
    hT = nc.dram_tensor("hT", [P, KD, B], _BF16, kind="ExternalInput").ap()
    hT8 = nc.dram_tensor("hT8", [P, KD, B], _F8, kind="ExternalInput").ap()
    tT = nc.dram_tensor("tT", [P, KD, B], _BF16, kind="ExternalInput").ap()
    tl = nc.dram_tensor("tl", [B, D], _F32, kind="ExternalInput").ap()
    w = nc.dram_tensor("w", [HSL, P, KD, D], _BF16, kind="ExternalInput").ap()
    w8 = nc.dram_tensor("w8", [len(FP8_HS), P, KD, D], _F8,
                        kind="ExternalInput").ap()
    vc = nc.dram_tensor("vc", [P, KC, HSL], _BF16, kind="ExternalInput").ap()
    ub = nc.dram_tensor("ub", [P, HSL], _F32, kind="ExternalInput").ap()
    br = nc.dram_tensor("br", [P, 1], _F32, kind="ExternalInput").ap()
    out = nc.dram_tensor("scores_part", [P, BT], _F32, kind="ExternalOutput").ap()

    with tile.TileContext(nc) as tc, ExitStack() as ctx:
        const = ctx.enter_context(tc.tile_pool(name="const", bufs=1))
        wp = ctx.enter_context(tc.tile_pool(name="w", bufs=4))
        psp = ctx.enter_context(tc.tile_pool(name="ps", bufs=6, space="PSUM"))
        lps = ctx.enter_context(tc.tile_pool(name="lps", bufs=2, space="PSUM"))
        scr = ctx.enter_context(tc.tile_pool(name="scr", bufs=2))

        # --- DMAs in priority order: compute start gates on W[0] + hT only.
        w_tiles = {}

        def load_w(h):
            w_t = wp.tile([P, KD, D], _BF16, name="wt")
            nc.sync.dma_start(w_t[:], w[h])
            return w_t

        w_tiles[0] = load_w(0)

        # head^T / tail as interleaved 2-batch-tile chunks: the h=0 matmul
        # quad for tile bt only gates on its 256KB hT chunk, so compute
        # starts early and h=0/h=1 run compute-paced (each dma_start costs
        # ~740ns of Sync-engine issue time, so 1-bt chunks would pace the
        # PE at the trigger rate instead).
        hT_t = const.tile([P, KD, B], _BF16)
        tT_t = const.tile([P, KD, B], _BF16)
        tl_t = const.tile([P, BT, D], _F32)
        # bt=0 alone first (halves the bytes gating the first matmul)
        nc.sync.dma_start(hT_t[:, :, 0:P], hT[:, :, 0:P])
        nc.sync.dma_start(tl_t[:, 0:1, :], tl[0:P, :].rearrange(
            "(t p) d -> p t d", p=P))
        nc.sync.dma_start(hT_t[:, :, P:2 * P], hT[:, :, P:2 * P])
        nc.sync.dma_start(tl_t[:, 1:2, :], tl[P:2 * P, :].rearrange(
            "(t p) d -> p t d", p=P))
        w_tiles[1] = load_w(1)
        for bt in range(2, BT, 2):
            sl = slice(bt * P, (bt + 2) * P)
            nc.sync.dma_start(hT_t[:, :, sl], hT[:, :, sl])
            nc.sync.dma_start(tl_t[:, bt:bt + 2, :], tl[sl, :].rearrange(
                "(t p) d -> p t d", p=P))
        # tail^T is only needed by the linear phase (inserted at h=5,
        # ~150us in): one bulk DMA, issued after the streams above. Same
        # for the fp8 head copy (first used at h=8) and fp8 W tiles.
        nc.sync.dma_start(tT_t[:], tT[:])
        hT8_t = const.tile([P, KD, B], _F8)
        nc.sync.dma_start(hT8_t[:], hT8[:])
        w8_tiles = []
        for i in range(len(FP8_HS)):
            w8_t = const.tile([P, KD, D], _F8, name=f"w8_{i}")
            nc.sync.dma_start(w8_t[:], w8[i])
            w8_tiles.append(w8_t)

        vc_t = const.tile([P, KC, HSL], _BF16)
        nc.sync.dma_start(vc_t[:], vc[:])
        ub_t = const.tile([P, HSL], _F32)
        nc.sync.dma_start(ub_t[:], ub[:, :])
        br_t = const.tile([P, 1], _F32)
        nc.sync.dma_start(br_t[:], br[:, :])
        w_tiles[2] = load_w(2)
        w_tiles[3] = load_w(3)

        bil_t = const.tile([P, BT, HSL], _F32)   # pure bilinear, b-major
        linb_t = const.tile([P, BT, HSL], _F32)  # linear + bias, b-major
        scores_t = const.tile([P, BT], _F32)

        lsp = ctx.enter_context(tc.tile_pool(name="lst", bufs=2))

        def lin_phase():
            # col-tiled: 4 batch-512 tiles accumulate concurrently in the
            # four 32-partition column groups of one PSUM bank. pl[32j+c, n]
            # = lin^T[h=c, b=(rnd*4+j)*512+n]; per-col-group accumulation
            # groups are independent (zero regions are per-partition-range).
            for rnd in range(2):
                pl = lps.tile([P, 512], _F32, name="pl")
                for kc in range(KC):
                    for j in range(4):
                        b512 = rnd * 4 + j
                        if kc < KD:
                            rhs = hT_t[:, kc, b512 * 512:(b512 + 1) * 512]
                        else:
                            rhs = tT_t[:, kc - KD, b512 * 512:(b512 + 1) * 512]
                        nc.tensor.matmul(
                            pl[32 * j:32 * (j + 1), :], vc_t[:, kc, :], rhs,
                            start=(kc == 0), stop=(kc == KC - 1),
                            tile_position=(0, 32 * j),
                        )
                # bias add in place (per-partition scalar = b_R tiled 4x)
                nc.vector.tensor_scalar_add(pl[:], pl[:], br_t[:])
                # transpose all 64 32x32 blocks in one DVE op
                lin_stage = lsp.tile([P, 512], _F32, name="lst")
                nc.vector.transpose(lin_stage[:], pl[:])
                # scatter to b-major linb_t: element [32j+r, 32(4q+m)+c] is
                # lin[h=c, b=(rnd*4+j)*512+32(4q+m)+r] -> partition 32m+r,
                # bt=(rnd*4+j)*4+q. One SBUF->SBUF DMA per (j, m).
                for j in range(4):
                    blk = lin_stage[32 * j:32 * (j + 1), :].rearrange(
                        "p (q m c) -> p q m c", q=4, m=4
                    )
                    for m in range(4):
                        dst = linb_t[32 * m:32 * (m + 1),
                                     (rnd * 4 + j) * 4:(rnd * 4 + j) * 4 + 4, :]
                        nc.sync.dma_start(dst, blk[:, :, m, :])

        # --- Phase 2: per h: T_h = head @ W_h ; bil[:, h] = rowsum(T_h * tail)
        # On the last h, phase-3 relu prep is interleaved per batch tile.
        s2p = ctx.enter_context(tc.tile_pool(name="s2", bufs=2))

        def _udot(bt):
            # scores_part[b] = relu(bil + lin)[b, :] @ u_slice
            s2_t = s2p.tile([P, HSL], _F32, name="s2")
            nc.vector._custom_dve(
                TENSOR_TENSOR_REDUCE,
                out=s2_t[:],
                in0=bil_t[:, bt, :],
                in1=ub_t[:],
                s0=0.0,
                s1=1.0,
                accum_out=scores_t[:, bt:bt + 1],
            )

        def quad(h, bt, w_t, fp8=False):
            ps_t = psp.tile([P, D], _F32, name="ps")
            if fp8:
                # DoubleRow: K=256 per matmul via [Ki, 2, dim] pair APs
                for c in range(2):
                    nc.tensor.matmul(
                        ps_t[:],
                        hT8_t[:, 2 * c:2 * c + 2, bt * P:(bt + 1) * P],
                        w_t[:, 2 * c:2 * c + 2, :],
                        start=(c == 0),
                        stop=(c == 1),
                        perf_mode=mybir.MatmulPerfMode.DoubleRow,
                    )
            else:
                for k in range(KD):
                    nc.tensor.matmul(
                        ps_t[:],
                        hT_t[:, k, bt * P:(bt + 1) * P],
                        w_t[:, k, :],
                        start=(k == 0),
                        stop=(k == KD - 1),
                    )
            s_t = scr.tile([P, D], _F32, name="s")
            nc.vector._custom_dve(
                TENSOR_TENSOR_REDUCE,
                out=s_t[:],
                in0=ps_t[:],
                in1=tl_t[:, bt, :],
                s0=0.0,
                s1=1.0,
                accum_out=bil_t[:, bt, h:h + 1],
            )

        # h=0 and h=1 interleaved per bt: during this window the hT/tl
        # streams are still landing, so give the PE 2 quads per arriving tile.
        for bt in range(BT):
            quad(0, bt, w_tiles[0])
            quad(1, bt, w_tiles[1])
        w_tiles.pop(0)
        w_tiles.pop(1)

        bf16_seq = [h for h in range(2, HSL) if h not in FP8_HS]
        pre = 2  # bf16_seq[0:2] == (2, 3) already in flight
        for h in range(2, HSL):
            fp8 = h in FP8_HS
            if fp8:
                w_t = w8_tiles[FP8_HS.index(h)]
            else:
                if pre < len(bf16_seq):
                    w_tiles[bf16_seq[pre]] = load_w(bf16_seq[pre])
                    pre += 1
                w_t = w_tiles.pop(h)
            for bt in range(BT):
                quad(h, bt, w_t, fp8=fp8)
                if h == HSL - 1:
                    # in-place: bil := relu(bil + lin)
                    nc.vector.tensor_add(
                        bil_t[:, bt, :], bil_t[:, bt, :], linb_t[:, bt, :]
                    )
                    nc.scalar.activation(
                        bil_t[:, bt, :], bil_t[:, bt, :],
                        mybir.ActivationFunctionType.Relu,
                    )
                    if bt >= 1:
                        _udot(bt - 1)
                    if bt - 1 == 15:
                        nc.sync.dma_start(out[:, 0:16], scores_t[:, 0:16])
                    if bt - 1 == 30:
                        nc.sync.dma_start(out[:, 16:31], scores_t[:, 16:31])
            if h == LIN_AT_H - 1:
                lin_phase()

        _udot(BT - 1)
        nc.sync.dma_start(out[:, 31:BT], scores_t[:, 31:BT])

    nc.compile()
    return nc


def _get_nc():
    global _NC_CACHE
    if _NC_CACHE is None:
        _NC_CACHE = _build_nc()
    return _NC_CACHE


def kernel(head_embeddings, relation_embeddings, tail_embeddings, W_R, V_R, u_R, b_R):
    head = np.asarray(head_embeddings, dtype=np.float32)
    tail = np.asarray(tail_embeddings, dtype=np.float32)
    W = np.asarray(W_R, dtype=np.float32)
    V = np.asarray(V_R, dtype=np.float32)
    u = np.asarray(u_R, dtype=np.float32)
    b = np.asarray(b_R, dtype=np.float32)

    bf = ml_dtypes.bfloat16
    f8 = ml_dtypes.float8_e4m3fn
    # [D, B] -> [P, KD, B]: partition p holds row k*128+p of the transpose
    hTr = head.T.reshape(KD, P, B).transpose(1, 0, 2)
    hTa = np.ascontiguousarray(hTr.astype(bf))
    hT8a = np.ascontiguousarray(hTr.astype(f8))
    tTa = np.ascontiguousarray(
        tail.T.reshape(KD, P, B).transpose(1, 0, 2).astype(bf))

    in_maps = []
    for c in range(NCORES):
        hs = slice(c * HSL, (c + 1) * HSL)
        # W[hs]: [HSL, D, D] -> [HSL, P, KD, D]
        wr = W[hs].reshape(HSL, KD, P, D).transpose(0, 2, 1, 3)
        wa = np.ascontiguousarray(wr.astype(bf))
        w8a = np.ascontiguousarray(wr[list(FP8_HS)].astype(f8))
        # V[hs].T: [2D, HSL] -> [P, KC, HSL]
        vca = np.ascontiguousarray(
            V[hs].T.reshape(KC, P, HSL).transpose(1, 0, 2).astype(bf))
        in_maps.append({
            "hT": hTa,
            "hT8": hT8a,
            "tT": tTa,
            "tl": tail,
            "w": wa,
            "w8": w8a,
            "vc": vca,
            "ub": np.ascontiguousarray(np.broadcast_to(u[hs], (P, HSL))),
            "br": np.ascontiguousarray(np.tile(b[hs], 4).reshape(P, 1)),
        })

    nc = _get_nc()
    trace = bool(int(os.environ.get("BILINEAR_TRACE", "0")))
    res = bass_utils.run_bass_kernel_spmd(
        nc, in_maps, core_ids=list(range(NCORES)), trace=trace
    )
    global LAST_RESULT
    LAST_RESULT = res
    if trace:
        print(f"HW exec time: {res.exec_time_ns} ns")
        if res.instructions_and_trace:
            print(f"trace: {res.instructions_and_trace[1]}")

    acc = np.zeros(B, dtype=np.float64)
    for c in range(NCORES):
        part = res.results[c]["scores_part"]  # [P, BT]
        acc += part.T.reshape(-1).astype(np.float64)
    return acc.astype(np.float32)
